# revision 16
# baseline (speedup 1.0000x reference)
"""Newton-Schulz matrix square root (nn_ASQRT) on 8 TRN2 NeuronCores.

Input  A: [32, 32, 128, 128] fp32 SPD matrices.
Output sA matching the 5-step coupled Newton-Schulz reference.

Data-parallel: 1024 matrices sharded 128 per core across 8 cores.

Per-matrix algebra (all iterates commute, symmetric):
    V0 = 0.5*A/nrm ; T0 = 1.5I - V0 ; Yh0 = V0
    u_n  = T_n^2 - 1.5 T_n          (note u0 = V0^2 - 1.5 V0)
    T_{n+1} = 1.5I + T_n u_n
    Yh_{n+1} = T_n Yh_n
    out = Yh_5 * 2*sqrt(nrm)

Sign trick at iter 0: u0 == -Yh1, so u0 is written straight into the
Yh slot of ty1 and the final scale is negated (no iter-0 Yh copy, no
T0 materialization; MM-A0/MM-B0 use V0 as stationary).

Matmul economics on TRN2: f32r is 1 cycle/row only at output free size
>= 256 (4 cyc/row below) and the HW verifier forbids mixing f32r with
bf16 in one matmul. All matmul tiles are therefore bf16 (1 cycle/row at
any width): MM-A = T.[T|Yh] 256-wide, MM-B = T.u 128-wide, psb stays one
PSUM bank. PSUM accumulation is f32; elementwise STT/copies read f32
PSUM and write bf16 SBUF, so each tensor is rounded once per hop.
Error model (numpy, bf16-RN + tf32 matmul-input rounding vs fp64):
~1.3e-2; measured HW f32r noise runs ~0.55x the model's tf32 part.

Emission is a staggered software pipeline, two stages per iteration
(A: MM-A + u-STT + Yh-copy ; B: MM-B + T'-STT) so every matmul's inputs
come from an earlier tick and no engine queue blocks on same-tick work.
PSUM: psa-tag 2x4KB + psb-tag 4x2KB (psn, psb0, psb1-3, psf) = 16KB.

GpSimd cannot touch PSUM on TRN2, so PSUM evacuation is DVE+ACT only:
every +1.5I (and iter-0's +1.5*u0) is folded into PSUM by const matmuls
so T' tiles are pure ACT copies; V0 (negated) and rowsq run on GpSimd;
u-STTs and the final scale run on DVE.

Engine budget per 4-matrix group/tick (ns):
  PE 3862 | DVE 3902 | ACT 4005 | GpS 2492
"""
import os
import sys

sys.path.insert(0, "/opt/trn_rl_repo")

from contextlib import ExitStack

import numpy as np

B_S, C_DIM, N = 32, 32, 128
NCORES = 8
NMAT = int(os.environ.get("ASQRT_NMAT", str((B_S * C_DIM) // NCORES)))
GRP = 4                         # matrices per fused op / PSUM tile
NUM_ITER = 5

_CACHE = {}
LAST_EXEC_NS = None


def const_inputs():
    import ml_dtypes

    ident = np.eye(N, dtype=np.float32)
    return {
        "c15b": (1.5 * ident).astype(ml_dtypes.bfloat16),
        "cm15b": (-1.5 * ident).astype(ml_dtypes.bfloat16),
        "cm15h": (-1.5 * ident).astype(np.float16),
        "identb": ident.astype(ml_dtypes.bfloat16),
    }


def _build(dt_mm_name: str):
    import concourse.bacc as bacc
    import concourse.tile as tile
    import concourse.mybir as mybir

    F32 = mybir.dt.float32
    F32R = mybir.dt.float32r
    BF16 = mybir.dt.bfloat16
    F16 = mybir.dt.float16
    AF = mybir.ActivationFunctionType
    ALU = mybir.AluOpType

    nc = bacc.Bacc(trn_type="TRN2", target_bir_lowering=False, debug=False)
    a = nc.dram_tensor("a", [NMAT, N, N], F32, kind="ExternalInput").ap()
    c15b = nc.dram_tensor("c15b", [N, N], BF16, kind="ExternalInput").ap()
    cm15b = nc.dram_tensor("cm15b", [N, N], BF16, kind="ExternalInput").ap()
    cm15h = nc.dram_tensor("cm15h", [N, N], F16, kind="ExternalInput").ap()
    identb = nc.dram_tensor("identb", [N, N], BF16, kind="ExternalInput").ap()
    o = nc.dram_tensor("o", [NMAT, N, N], F32, kind="ExternalOutput").ap()

    with tile.TileContext(nc) as tc, ExitStack() as ctx:
        cpool = ctx.enter_context(tc.tile_pool(name="consts", bufs=1))
        a_pool = ctx.enter_context(tc.tile_pool(name="a", bufs=10))
        v0_pool = ctx.enter_context(tc.tile_pool(name="v0", bufs=4))
        ty_pool = ctx.enter_context(tc.tile_pool(name="ty", bufs=18))
        u_pool = ctx.enter_context(tc.tile_pool(name="u", bufs=8))
        sq_pool = ctx.enter_context(tc.tile_pool(name="sq", bufs=4))
        out_pool = ctx.enter_context(tc.tile_pool(name="out", bufs=4))
        nrm_pool = ctx.enter_context(tc.tile_pool(name="nrm", bufs=6))
        s2_pool = ctx.enter_context(tc.tile_pool(name="s2p", bufs=16))
        psa_pool = ctx.enter_context(tc.tile_pool(name="psa", bufs=2, space="PSUM"))
        psb_pool = ctx.enter_context(tc.tile_pool(name="psb", bufs=4, space="PSUM"))

        c15bt = cpool.tile([N, N], BF16, tag="c15b")
        nc.sync.dma_start(c15bt[:], c15b)
        cm15bt = cpool.tile([N, N], BF16, tag="cm15b")
        nc.sync.dma_start(cm15bt[:], cm15b)
        cm15ht = cpool.tile([N, N], F16, tag="cm15h")
        nc.sync.dma_start(cm15ht[:], cm15h)
        idb = cpool.tile([N, N], BF16, tag="idb")
        nc.sync.dma_start(idb[:], identb)
        onest = cpool.tile([N, N], F32, tag="onest")
        nc.vector.memset(onest[:], 1.0)

        ngrp = NMAT // GRP

        def dup(ap2d):
            # [N, N] -> [N, 2, N] with stride-0 middle dim: 256-wide rhs
            return ap2d.unsqueeze(1).broadcast_to([N, 2, N])

        st = {}  # per-group state

        def emit_dma(g):
            base = g * GRP
            ag = a_pool.tile([N, GRP, N], F32, tag="aq", name=f"aq{base}")
            nc.sync.dma_start(
                ag[:], a[base : base + GRP].rearrange("b p f -> p b f")
            )
            st[g] = {"ag": ag}

        def emit_rowsq(g):
            base = g * GRP
            s = st[g]
            rsg = nrm_pool.tile([N, GRP], F32, tag="rs", name=f"rs{base}")
            sq = sq_pool.tile([N, GRP, N], F32, tag="sq", name=f"sq{base}")
            for j in range(GRP):
                # STT+accum is DVE-only (Pool lacks the accumulator opcode)
                nc.vector.scalar_tensor_tensor(
                    out=sq[:, j, :], in0=s["ag"][:, j, :], scalar=1.0,
                    in1=s["ag"][:, j, :], op0=ALU.mult, op1=ALU.mult,
                    accum_out=rsg[:, j : j + 1],
                )
            s["rsg"] = rsg

        def emit_norm(g):
            """PE norm broadcast + ACT/DVE scalar chain (V0 emitted later)."""
            base = g * GRP
            s = st[g]
            psn = psb_pool.tile([N, GRP], F32, tag="psb", name=f"psn{base}")
            nc.tensor.matmul(
                psn[:], lhsT=onest[:], rhs=s.pop("rsg"), start=True, stop=True
            )
            nrm2 = nrm_pool.tile([N, GRP], F32, tag="nrm2", name=f"nrm2{base}")
            nc.scalar.activation(nrm2[:], psn[:], AF.Sqrt, scale=4.0)  # 2*||A||
            s2 = s2_pool.tile([N, GRP], F32, tag="s2", name=f"s2{base}")
            nc.scalar.activation(s2[:], nrm2[:], AF.Sqrt, scale=2.0)   # 2*sqrt||A||
            rcp = nrm_pool.tile([N, GRP], F32, tag="rcp", name=f"rcp{base}")
            nc.vector.reciprocal(rcp[:], nrm2[:])                      # 0.5/||A||
            s["rcp"], s["s2"] = rcp, s2

        def emit_v0(g):
            base = g * GRP
            s = st[g]
            rcp = s.pop("rcp")
            v0 = v0_pool.tile([N, GRP, N], F16, tag="v0", name=f"v0{base}")
            for j in range(GRP):
                nc.gpsimd.tensor_tensor(
                    v0[:, j, :], s["ag"][:, j, :],
                    rcp[:, j : j + 1].broadcast_to([N, N]),
                    ALU.mult,
                )
            s.pop("ag")
            s["v0"] = v0

        def emit_it0A(g):
            base = g * GRP
            s = st[g]
            v0 = s["v0"]
            psa = psa_pool.tile([N, GRP, 2 * N], F32, tag="psa", name=f"psa{base}_0")
            for j in range(GRP):
                nc.tensor.matmul(
                    psa[:, j, :], lhsT=v0[:, j, :], rhs=dup(v0[:, j, :]),
                    start=True, stop=True,
                )
            ty = ty_pool.tile([N, GRP, 2 * N], F16, tag="ty", name=f"ty{base}_1")
            # u0 = V0^2 - 1.5 V0 -> ty1 Yh slot (== -Yh1, sign folded at out)
            nc.vector.scalar_tensor_tensor(
                out=ty[:, :, N:], in0=v0[:], scalar=-1.5,
                in1=psa[:, :, 0:N], op0=ALU.mult, op1=ALU.add,
            )
            s["ty"] = ty

        def emit_it0B(g):
            base = g * GRP
            s = st[g]
            v0, ty = s.pop("v0"), s["ty"]
            psb = psb_pool.tile([N, GRP, N], F32, tag="psb", name=f"psb{base}_0")
            # folds first (512-wide, one weight load each), then per-j MMs
            nc.tensor.matmul(  # psb = -1.5I on all j
                psb[:], lhsT=cm15bt[:],
                rhs=idb[:].unsqueeze(1).broadcast_to([N, GRP, N]),
                start=True, stop=False, skip_group_check=True,
            )
            nc.tensor.matmul(  # += -1.5 u0 on all j
                psb[:], lhsT=cm15ht[:], rhs=ty[:, :, N:],
                start=False, stop=False, skip_group_check=True,
            )
            for j in range(GRP):
                nc.tensor.matmul(  # += V0 u0  => psb = -T1
                    psb[:, j, :], lhsT=v0[:, j, :], rhs=ty[:, j, N:],
                    start=False, stop=True, skip_group_check=True,
                )
            nc.scalar.activation(ty[:, :, 0:N], psb[:], AF.Copy, scale=-1.0)

        def emit_itA(g, it):
            base = g * GRP
            s = st[g]
            ty = s["ty"]
            psa = psa_pool.tile(
                [N, GRP, 2 * N], F32, tag="psa", name=f"psa{base}_{it}"
            )
            for j in range(GRP):
                nc.tensor.matmul(
                    psa[:, j, :], lhsT=ty[:, j, 0:N], rhs=ty[:, j, :],
                    start=True, stop=True,
                )
            u = u_pool.tile([N, GRP, N], F16, tag="u", name=f"u{base}_{it}")
            if it < NUM_ITER - 2:
                nc.vector.scalar_tensor_tensor(
                    out=u[:], in0=ty[:, :, 0:N], scalar=-1.5,
                    in1=psa[:, :, 0:N], op0=ALU.mult, op1=ALU.add,
                )
            else:
                # u3 = 1.5 T - T^2 = -u: makes psb3 = -1.5I - T u = -T4 so
                # psf = (-T4)(-Yh4) = +Yh5 and the out scale is +2 sqrt(nrm)
                nc.vector.scalar_tensor_tensor(
                    out=u[:], in0=ty[:, :, 0:N], scalar=1.5,
                    in1=psa[:, :, 0:N], op0=ALU.mult, op1=ALU.subtract,
                )
            tyn = ty_pool.tile(
                [N, GRP, 2 * N], F16, tag="ty", name=f"ty{base}_{it + 1}"
            )
            nc.scalar.copy(tyn[:, :, N:], psa[:, :, N:])
            s["u"], s["tyn"] = u, tyn

        def emit_itB(g, it):
            base = g * GRP
            s = st[g]
            ty, tyn, u = s["ty"], s["tyn"], s.pop("u")
            psb = psb_pool.tile(
                [N, GRP, N], F32, tag="psb", name=f"psb{base}_{it}"
            )
            seed = c15bt if it < NUM_ITER - 2 else cm15bt
            nc.tensor.matmul(  # psb = +-1.5I on all j
                psb[:], lhsT=seed[:],
                rhs=idb[:].unsqueeze(1).broadcast_to([N, GRP, N]),
                start=True, stop=False, skip_group_check=True,
            )
            for j in range(GRP):
                nc.tensor.matmul(  # += T u  => psb = T' (it3: -T4)
                    psb[:, j, :], lhsT=ty[:, j, 0:N], rhs=u[:, j, :],
                    start=False, stop=True, skip_group_check=True,
                )
            nc.scalar.copy(tyn[:, :, 0:N], psb[:])
            s["ty"] = tyn
            del s["tyn"]

        def emit_finA(g):
            base = g * GRP
            s = st[g]
            ty = s.pop("ty")
            psf = psb_pool.tile([N, GRP, N], F32, tag="psb", name=f"psf{base}")
            for j in range(GRP):
                nc.tensor.matmul(
                    psf[:, j, :], lhsT=ty[:, j, 0:N], rhs=ty[:, j, N:],
                    start=True, stop=True,
                )
            s["psf"] = psf

        def emit_finB(g):
            base = g * GRP
            s = st.pop(g)
            psf, s2 = s["psf"], s["s2"]
            outg = out_pool.tile([N, GRP, N], F32, tag="outq", name=f"out{base}")
            # out = +2*sqrt(nrm) * psf  (signs cancel: psf = (-T4)(-Yh4))
            for j in range(GRP):
                nc.vector.tensor_scalar_mul(
                    outg[:, j, :], psf[:, j, :], s2[:, j : j + 1]
                )
            nc.sync.dma_start(
                o[base : base + GRP].rearrange("b p f -> p b f"), outg[:]
            )

        # --- staggered pipeline ------------------------------------------
        # offsets: dma@0 rowsq@1 norm@2 it0A@3 it0B@4 it1A@5 it1B@6
        #          it2A@7 it2B@8 it3A@9 it3B@10 finA@11 finB@12
        # Emission order within a tick shapes each engine's queue: finB
        # first (frees psf), then rowsq/norm scalars, the A-stages (psa
        # producers early), B-stages, finA, V0 late (GpS tail), dma last.
        DEPTH = 12

        def ok(g):
            return 0 <= g < ngrp

        for t in range(ngrp + DEPTH):
            if ok(t - 12):
                emit_finB(t - 12)
            if ok(t - 1):
                emit_rowsq(t - 1)
            if ok(t - 2):
                emit_norm(t - 2)
            if ok(t - 3):
                emit_it0A(t - 3)
            if ok(t - 5):
                emit_itA(t - 5, 1)
            if ok(t - 7):
                emit_itA(t - 7, 2)
            if ok(t - 9):
                emit_itA(t - 9, 3)
            if ok(t - 4):
                emit_it0B(t - 4)
            if ok(t - 6):
                emit_itB(t - 6, 1)
            if ok(t - 8):
                emit_itB(t - 8, 2)
            if ok(t - 10):
                emit_itB(t - 10, 3)
            if ok(t - 11):
                emit_finA(t - 11)
            if ok(t - 2):
                emit_v0(t - 2)
            if ok(t):
                emit_dma(t)

    nc.compile()
    return nc


def _get_nc():
    dt_mm = os.environ.get("ASQRT_DTYPE", "f32r")
    if dt_mm not in _CACHE:
        _CACHE[dt_mm] = _build(dt_mm)
    return _CACHE[dt_mm]


def kernel(A: np.ndarray) -> np.ndarray:
    global LAST_EXEC_NS
    from concourse.bass_utils import run_bass_kernel_spmd

    nc = _get_nc()
    A2 = np.ascontiguousarray(A.reshape(-1, N, N), dtype=np.float32)
    consts = const_inputs()
    in_maps = [
        {"a": A2[i * NMAT : (i + 1) * NMAT], **consts}
        for i in range(NCORES)
    ]
    trace = os.environ.get("ASQRT_TRACE", "0") == "1"
    res = run_bass_kernel_spmd(nc, in_maps, list(range(NCORES)), trace=trace)
    LAST_EXEC_NS = res.exec_time_ns
    out = np.concatenate([r["o"] for r in res.results], axis=0)
    return out.reshape(B_S, C_DIM, N, N)


if __name__ == "__main__":
    rng = np.random.default_rng(0)
    A = rng.standard_normal((B_S, C_DIM, N, N)).astype(np.float32)
    A = np.einsum("bcij,bckj->bcik", A, A) / N + 1e-3 * np.eye(N, dtype=np.float32)
    out = kernel(A)
    print("ok", out.shape, LAST_EXEC_NS)


# revision 18
# speedup vs baseline: 1.0126x; 1.0126x over previous
"""Newton-Schulz matrix square root (nn_ASQRT) on 8 TRN2 NeuronCores.

Input  A: [32, 32, 128, 128] fp32 SPD matrices.
Output sA matching the 5-step coupled Newton-Schulz reference.

Data-parallel: 1024 matrices sharded 128 per core across 8 cores.

Per-matrix algebra (all iterates commute, symmetric):
    V0 = 0.5*A/nrm ; T0 = 1.5I - V0 ; Yh0 = V0
    u_n  = T_n^2 - 1.5 T_n          (note u0 = V0^2 - 1.5 V0)
    T_{n+1} = 1.5I + T_n u_n
    Yh_{n+1} = T_n Yh_n
    out = Yh_5 * 2*sqrt(nrm)

Sign trick at iter 0: u0 == -Yh1, so u0 is written straight into the
Yh slot of ty1 and the final scale is negated (no iter-0 Yh copy, no
T0 materialization; MM-A0/MM-B0 use V0 as stationary).

Matmul economics on TRN2: f32r is 1 cycle/row only at output free size
>= 256 (4 cyc/row below) and the HW verifier forbids mixing f32r with
bf16 in one matmul. All matmul tiles are therefore bf16 (1 cycle/row at
any width): MM-A = T.[T|Yh] 256-wide, MM-B = T.u 128-wide, psb stays one
PSUM bank. PSUM accumulation is f32; elementwise STT/copies read f32
PSUM and write bf16 SBUF, so each tensor is rounded once per hop.
Error model (numpy, bf16-RN + tf32 matmul-input rounding vs fp64):
~1.3e-2; measured HW f32r noise runs ~0.55x the model's tf32 part.

Emission is a staggered software pipeline, two stages per iteration
(A: MM-A + u-STT + Yh-copy ; B: MM-B + T'-STT) so every matmul's inputs
come from an earlier tick and no engine queue blocks on same-tick work.
PSUM: psa-tag 2x4KB + psb-tag 4x2KB (psn, psb0, psb1-3, psf) = 16KB.

GpSimd cannot touch PSUM on TRN2, so PSUM evacuation is DVE+ACT only:
every +1.5I (and iter-0's +1.5*u0) is folded into PSUM by const matmuls
so T' tiles are pure ACT copies; V0 (negated) and rowsq run on GpSimd;
u-STTs and the final scale run on DVE.

Engine budget per 4-matrix group/tick (ns):
  PE 3862 | DVE 3902 | ACT 4005 | GpS 2492
"""
import os
import sys

sys.path.insert(0, "/opt/trn_rl_repo")

from contextlib import ExitStack

import numpy as np

B_S, C_DIM, N = 32, 32, 128
NCORES = 8
NMAT = int(os.environ.get("ASQRT_NMAT", str((B_S * C_DIM) // NCORES)))
GRP = 4                         # matrices per fused op / PSUM tile
NUM_ITER = 5

_CACHE = {}
LAST_EXEC_NS = None


def const_inputs():
    import ml_dtypes

    ident = np.eye(N, dtype=np.float32)
    return {
        "c15b": (1.5 * ident).astype(ml_dtypes.bfloat16),
        "cm15b": (-1.5 * ident).astype(ml_dtypes.bfloat16),
        "cm15h": (-1.5 * ident).astype(np.float16),
        "identb": ident.astype(ml_dtypes.bfloat16),
        "idz": np.concatenate([ident, 0 * ident], axis=1).astype(
            ml_dtypes.bfloat16
        ),
    }


def _build(dt_mm_name: str):
    import concourse.bacc as bacc
    import concourse.tile as tile
    import concourse.mybir as mybir

    F32 = mybir.dt.float32
    F32R = mybir.dt.float32r
    BF16 = mybir.dt.bfloat16
    F16 = mybir.dt.float16
    AF = mybir.ActivationFunctionType
    ALU = mybir.AluOpType

    nc = bacc.Bacc(trn_type="TRN2", target_bir_lowering=False, debug=False)
    a = nc.dram_tensor("a", [NMAT, N, N], F32, kind="ExternalInput").ap()
    c15b = nc.dram_tensor("c15b", [N, N], BF16, kind="ExternalInput").ap()
    cm15b = nc.dram_tensor("cm15b", [N, N], BF16, kind="ExternalInput").ap()
    cm15h = nc.dram_tensor("cm15h", [N, N], F16, kind="ExternalInput").ap()
    identb = nc.dram_tensor("identb", [N, N], BF16, kind="ExternalInput").ap()
    idz = nc.dram_tensor("idz", [N, 2 * N], BF16, kind="ExternalInput").ap()
    o = nc.dram_tensor("o", [NMAT, N, N], F32, kind="ExternalOutput").ap()

    with tile.TileContext(nc) as tc, ExitStack() as ctx:
        cpool = ctx.enter_context(tc.tile_pool(name="consts", bufs=1))
        a_pool = ctx.enter_context(tc.tile_pool(name="a", bufs=10))
        v0_pool = ctx.enter_context(tc.tile_pool(name="v0", bufs=4))
        ty_pool = ctx.enter_context(tc.tile_pool(name="ty", bufs=18))
        sq_pool = ctx.enter_context(tc.tile_pool(name="sq", bufs=4))
        out_pool = ctx.enter_context(tc.tile_pool(name="out", bufs=4))
        nrm_pool = ctx.enter_context(tc.tile_pool(name="nrm", bufs=6))
        s2_pool = ctx.enter_context(tc.tile_pool(name="s2p", bufs=16))
        psa_pool = ctx.enter_context(tc.tile_pool(name="psa", bufs=4, space="PSUM"))
        psb_pool = ctx.enter_context(tc.tile_pool(name="psb", bufs=2, space="PSUM"))

        c15bt = cpool.tile([N, N], BF16, tag="c15b")
        nc.sync.dma_start(c15bt[:], c15b)
        cm15bt = cpool.tile([N, N], BF16, tag="cm15b")
        nc.sync.dma_start(cm15bt[:], cm15b)
        cm15ht = cpool.tile([N, N], F16, tag="cm15h")
        nc.sync.dma_start(cm15ht[:], cm15h)
        idb = cpool.tile([N, N], BF16, tag="idb")
        nc.sync.dma_start(idb[:], identb)
        idzt = cpool.tile([N, 2 * N], BF16, tag="idz")
        nc.sync.dma_start(idzt[:], idz)
        onest = cpool.tile([N, N], F32, tag="onest")
        nc.vector.memset(onest[:], 1.0)

        ngrp = NMAT // GRP

        def dup(ap2d):
            # [N, N] -> [N, 2, N] with stride-0 middle dim: 256-wide rhs
            return ap2d.unsqueeze(1).broadcast_to([N, 2, N])

        st = {}  # per-group state

        def emit_dma(g):
            base = g * GRP
            ag = a_pool.tile([N, GRP, N], F32, tag="aq", name=f"aq{base}")
            nc.sync.dma_start(
                ag[:], a[base : base + GRP].rearrange("b p f -> p b f")
            )
            st[g] = {"ag": ag}

        def emit_rowsq(g):
            base = g * GRP
            s = st[g]
            rsg = nrm_pool.tile([N, GRP], F32, tag="rs", name=f"rs{base}")
            sq = sq_pool.tile([N, GRP, N], F32, tag="sq", name=f"sq{base}")
            for j in range(GRP):
                # STT+accum is DVE-only (Pool lacks the accumulator opcode)
                nc.vector.scalar_tensor_tensor(
                    out=sq[:, j, :], in0=s["ag"][:, j, :], scalar=1.0,
                    in1=s["ag"][:, j, :], op0=ALU.mult, op1=ALU.mult,
                    accum_out=rsg[:, j : j + 1],
                )
            s["rsg"] = rsg

        def emit_norm(g):
            """PE norm broadcast + ACT/DVE scalar chain (V0 emitted later)."""
            base = g * GRP
            s = st[g]
            psn = psa_pool.tile([N, GRP], F32, tag="psa", name=f"psn{base}")
            nc.tensor.matmul(
                psn[:], lhsT=onest[:], rhs=s.pop("rsg"), start=True, stop=True
            )
            nrm2 = nrm_pool.tile([N, GRP], F32, tag="nrm2", name=f"nrm2{base}")
            nc.scalar.activation(nrm2[:], psn[:], AF.Sqrt, scale=4.0)  # 2*||A||
            s2 = s2_pool.tile([N, GRP], F32, tag="s2", name=f"s2{base}")
            nc.scalar.activation(s2[:], nrm2[:], AF.Sqrt, scale=2.0)   # 2*sqrt||A||
            rcp = nrm_pool.tile([N, GRP], F32, tag="rcp", name=f"rcp{base}")
            nc.vector.reciprocal(rcp[:], nrm2[:])                      # 0.5/||A||
            s["rcp"], s["s2"] = rcp, s2

        def emit_v0(g):
            base = g * GRP
            s = st[g]
            rcp = s.pop("rcp")
            v0 = v0_pool.tile([N, GRP, N], F16, tag="v0", name=f"v0{base}")
            for j in range(GRP):
                nc.gpsimd.tensor_tensor(
                    v0[:, j, :], s["ag"][:, j, :],
                    rcp[:, j : j + 1].broadcast_to([N, N]),
                    ALU.mult,
                )
            s.pop("ag")
            s["v0"] = v0

        def emit_it0A(g):
            base = g * GRP
            s = st[g]
            v0 = s["v0"]
            psa = psa_pool.tile([N, GRP, N], F32, tag="psa", name=f"psa{base}_0")
            for j in range(GRP):
                nc.tensor.matmul(
                    psa[:, j, :], lhsT=v0[:, j, :], rhs=v0[:, j, :],
                    start=True, stop=True,
                )
            # ty layout: [T | u | Ytilde] (3N wide)
            ty = ty_pool.tile([N, GRP, 3 * N], F16, tag="ty", name=f"ty{base}_1")
            # u0 = V0^2 - 1.5 V0 -> ty1 Y-section (== -Yh1, sign folded at out)
            nc.vector.scalar_tensor_tensor(
                out=ty[:, :, 2 * N :], in0=v0[:], scalar=-1.5,
                in1=psa[:], op0=ALU.mult, op1=ALU.add,
            )
            s["ty"] = ty

        def emit_it0B(g):
            base = g * GRP
            s = st[g]
            v0, ty = s.pop("v0"), s["ty"]
            psb = psa_pool.tile([N, GRP, N], F32, tag="psa", name=f"psb{base}_0")
            # folds first (512-wide, one weight load each), then per-j MMs
            nc.tensor.matmul(  # psb = -1.5I on all j
                psb[:], lhsT=cm15bt[:],
                rhs=idb[:].unsqueeze(1).broadcast_to([N, GRP, N]),
                start=True, stop=False, skip_group_check=True,
            )
            nc.tensor.matmul(  # += -1.5 u0 on all j
                psb[:], lhsT=cm15ht[:], rhs=ty[:, :, 2 * N :],
                start=False, stop=False, skip_group_check=True,
            )
            for j in range(GRP):
                nc.tensor.matmul(  # += V0 u0  => psb = -T1
                    psb[:, j, :], lhsT=v0[:, j, :], rhs=ty[:, j, 2 * N :],
                    start=False, stop=True, skip_group_check=True,
                )
            nc.scalar.activation(ty[:, :, 0:N], psb[:], AF.Copy, scale=-1.0)

        def emit_itA(g, it):
            base = g * GRP
            s = st[g]
            ty = s["ty"]
            psa = psa_pool.tile(
                [N, GRP, N], F32, tag="psa", name=f"psa{base}_{it}"
            )
            for j in range(GRP):
                nc.tensor.matmul(
                    psa[:, j, :], lhsT=ty[:, j, 0:N], rhs=ty[:, j, 0:N],
                    start=True, stop=True,
                )
            if it < NUM_ITER - 2:
                nc.vector.scalar_tensor_tensor(
                    out=ty[:, :, N : 2 * N], in0=ty[:, :, 0:N], scalar=-1.5,
                    in1=psa[:], op0=ALU.mult, op1=ALU.add,
                )
            else:
                # u3 = 1.5 T - T^2 = -u: makes psb3 = -1.5I - T u = -T4 so
                # psf = (-T4)(-Yh4) = +Yh5 and the out scale is +2 sqrt(nrm)
                nc.vector.scalar_tensor_tensor(
                    out=ty[:, :, N : 2 * N], in0=ty[:, :, 0:N], scalar=1.5,
                    in1=psa[:], op0=ALU.mult, op1=ALU.subtract,
                )
            tyn = ty_pool.tile(
                [N, GRP, 3 * N], F16, tag="ty", name=f"ty{base}_{it + 1}"
            )
            s["tyn"] = tyn

        def emit_itB(g, it):
            base = g * GRP
            s = st[g]
            ty, tyn = s["ty"], s["tyn"]
            psb = psb_pool.tile(
                [N, GRP, 2 * N], F32, tag="psb", name=f"psb{base}_{it}"
            )
            seed = c15bt if it < NUM_ITER - 2 else cm15bt
            for h in range(2):  # seed halves: matmul out must stay in-bank
                nc.tensor.matmul(  # psb = [+-1.5I | 0] on j pair
                    psb[:, 2 * h : 2 * h + 2, :], lhsT=seed[:],
                    rhs=idzt[:].unsqueeze(1).broadcast_to([N, 2, 2 * N]),
                    start=True, stop=False, skip_group_check=True,
                )
            for j in range(GRP):
                nc.tensor.matmul(  # += T.[u|Y] => psb = [T' | Y']
                    psb[:, j, :], lhsT=ty[:, j, 0:N], rhs=ty[:, j, N:],
                    start=False, stop=True, skip_group_check=True,
                )
            # one copy: T' -> tyn[0:N], Y' -> tyn[2N:3N] (2-chunk out AP)
            tyn_tu = tyn[:, :, 0 : 3 * N].rearrange(
                "p b (c n) -> p b c n", c=3
            )[:, :, 0::2, :]
            nc.scalar.copy(tyn_tu, psb[:])
            s["ty"] = tyn
            del s["tyn"]

        def emit_finA(g):
            base = g * GRP
            s = st[g]
            ty = s.pop("ty")
            psf = psa_pool.tile([N, GRP, N], F32, tag="psa", name=f"psf{base}")
            for j in range(GRP):
                nc.tensor.matmul(
                    psf[:, j, :], lhsT=ty[:, j, 0:N], rhs=ty[:, j, 2 * N :],
                    start=True, stop=True,
                )
            s["psf"] = psf

        def emit_finB(g):
            base = g * GRP
            s = st.pop(g)
            psf, s2 = s["psf"], s["s2"]
            outg = out_pool.tile([N, GRP, N], F32, tag="outq", name=f"out{base}")
            # out = +2*sqrt(nrm) * psf  (signs cancel: psf = (-T4)(-Yh4))
            for j in range(GRP):
                nc.vector.tensor_scalar_mul(
                    outg[:, j, :], psf[:, j, :], s2[:, j : j + 1]
                )
            nc.sync.dma_start(
                o[base : base + GRP].rearrange("b p f -> p b f"), outg[:]
            )

        # --- staggered pipeline ------------------------------------------
        # offsets: dma@0 rowsq@1 norm@2 it0A@3 it0B@4 it1A@5 it1B@6
        #          it2A@7 it2B@8 it3A@9 it3B@10 finA@11 finB@12
        # Emission order within a tick shapes each engine's queue: finB
        # first (frees psf), then rowsq/norm scalars, the A-stages (psa
        # producers early), B-stages, finA, V0 late (GpS tail), dma last.
        DEPTH = 12

        def ok(g):
            return 0 <= g < ngrp

        for t in range(ngrp + DEPTH):
            if ok(t - 12):
                emit_finB(t - 12)
            if ok(t - 1):
                emit_rowsq(t - 1)
            if ok(t - 2):
                emit_norm(t - 2)
            if ok(t - 3):
                emit_it0A(t - 3)
            if ok(t - 5):
                emit_itA(t - 5, 1)
            if ok(t - 7):
                emit_itA(t - 7, 2)
            if ok(t - 9):
                emit_itA(t - 9, 3)
            if ok(t - 4):
                emit_it0B(t - 4)
            if ok(t - 6):
                emit_itB(t - 6, 1)
            if ok(t - 8):
                emit_itB(t - 8, 2)
            if ok(t - 10):
                emit_itB(t - 10, 3)
            if ok(t - 11):
                emit_finA(t - 11)
            if ok(t - 2):
                emit_v0(t - 2)
            if ok(t):
                emit_dma(t)

    nc.compile()
    return nc


def _get_nc():
    dt_mm = os.environ.get("ASQRT_DTYPE", "f32r")
    if dt_mm not in _CACHE:
        _CACHE[dt_mm] = _build(dt_mm)
    return _CACHE[dt_mm]


def kernel(A: np.ndarray) -> np.ndarray:
    global LAST_EXEC_NS
    from concourse.bass_utils import run_bass_kernel_spmd

    nc = _get_nc()
    A2 = np.ascontiguousarray(A.reshape(-1, N, N), dtype=np.float32)
    consts = const_inputs()
    in_maps = [
        {"a": A2[i * NMAT : (i + 1) * NMAT], **consts}
        for i in range(NCORES)
    ]
    trace = os.environ.get("ASQRT_TRACE", "0") == "1"
    res = run_bass_kernel_spmd(nc, in_maps, list(range(NCORES)), trace=trace)
    LAST_EXEC_NS = res.exec_time_ns
    out = np.concatenate([r["o"] for r in res.results], axis=0)
    return out.reshape(B_S, C_DIM, N, N)


if __name__ == "__main__":
    rng = np.random.default_rng(0)
    A = rng.standard_normal((B_S, C_DIM, N, N)).astype(np.float32)
    A = np.einsum("bcij,bckj->bcik", A, A) / N + 1e-3 * np.eye(N, dtype=np.float32)
    out = kernel(A)
    print("ok", out.shape, LAST_EXEC_NS)
